# revision 4
# baseline (speedup 1.0000x reference)
"""DifferenceAwareAggregator — Bass/Tile kernel, data-parallel on 8 NeuronCores.

Sharding: batch dim (B=8192) split across 8 cores (1024 centers each);
small projection weights replicated. Each core runs a 5-phase Tile kernel:

  PI   pre = Hn @ W1s - Hc @ W1b   (feature-major inputs pre-packed on host,
       so no on-chip input transposes); LN stats fused into the PSUM
       evacuation (scalar_tensor_tensor accum_out) + ACT Square accum_out;
       t spilled to HBM in bf16.
  PII  batched rsqrt of the LN variances (one sqrt-set excursion).
  PIII hn = Gelu(t*rs - mu*rs) fused on ScalarE; PE-transpose hn; K = hn@Wk;
       scores = head-reduce(Q * K).
  PIV  batched masked softmax over all 8 center-tiles (one exp-set excursion).
  PV   V = hn@Wv; prod = V * attn; ctx accumulated over neighbors in PSUM
       via identity-matmul; out = ctx @ Wo.

Algebra: concat([h_n, h_n - h_c]) @ W1 == h_n @ (W1top+W1bot) - h_c @ W1bot.
bk drops out of softmax (per-(b,h) constant shift). 1/sqrt(d) folded into Wq.
"""

import sys

import numpy as np
import ml_dtypes

_TRN = "/opt/trn_rl_repo"
if _TRN not in sys.path:
    sys.path.insert(0, _TRN)

bf16 = ml_dtypes.bfloat16

M = 8          # cores
B = 8192
N = 32         # neighbors
H = 512
BL = B // M    # centers per core
P = 128        # partitions
CT = BL // P   # center tiles per core
NHEAD = 8
DH = H // NHEAD
NEG = -30000.0
LN_EPS = 1e-5

_CACHE: dict = {}


def _build_nc():
    import concourse.mybir as mybir
    import concourse.tile as tile
    from concourse import bacc
    from concourse.masks import make_identity

    f32 = mybir.dt.float32
    bf = mybir.dt.bfloat16
    Alu = mybir.AluOpType
    Act = mybir.ActivationFunctionType
    X = mybir.AxisListType.X

    nc = bacc.Bacc()
    hnf_d = nc.dram_tensor("hnf", [CT, H, N, P], bf, kind="ExternalInput")
    hcf_d = nc.dram_tensor("hcf", [H, BL], bf, kind="ExternalInput")
    mb_d = nc.dram_tensor("mb", [CT, P, N], f32, kind="ExternalInput")
    w1s_d = nc.dram_tensor("w1s", [H, H], bf, kind="ExternalInput")
    w1b_d = nc.dram_tensor("w1b", [H, H], bf, kind="ExternalInput")
    wq_d = nc.dram_tensor("wq", [H, H], bf, kind="ExternalInput")
    wk_d = nc.dram_tensor("wk", [H, H], bf, kind="ExternalInput")
    wv_d = nc.dram_tensor("wv", [H, H], bf, kind="ExternalInput")
    wo_d = nc.dram_tensor("wo", [H, H], bf, kind="ExternalInput")
    out_d = nc.dram_tensor("out", [BL, H], f32, kind="ExternalOutput")

    hnf_re = hnf_d.rearrange("ct (fc p) n c -> ct p fc n c", p=P)

    from contextlib import ExitStack

    with tile.TileContext(nc) as tc:
        with ExitStack() as stack:
            ec = stack.enter_context
            singles = ec(tc.tile_pool(name="singles", bufs=1))
            dram = ec(tc.tile_pool(name="dram", bufs=1, space="DRAM"))
            zbp = ec(tc.tile_pool(name="zb", bufs=2))
            hnin = ec(tc.tile_pool(name="hnin", bufs=3))
            tp = ec(tc.tile_pool(name="tp", bufs=3))
            sqs = ec(tc.tile_pool(name="sqs", bufs=2))
            trd = ec(tc.tile_pool(name="trd", bufs=3))
            hnp = ec(tc.tile_pool(name="hnp", bufs=3))
            hfs = ec(tc.tile_pool(name="hfs", bufs=3))
            pkp = ec(tc.tile_pool(name="pk", bufs=3))
            hfr = ec(tc.tile_pool(name="hfr", bufs=3))
            axp = ec(tc.tile_pool(name="axp", bufs=3))
            pvp = ec(tc.tile_pool(name="pv", bufs=3))
            tail = ec(tc.tile_pool(name="tail", bufs=2))
            mmps = ec(tc.tile_pool(name="mmps", bufs=3, space="PSUM"))
            trps = ec(tc.tile_pool(name="trps", bufs=2, space="PSUM"))
            ctxps = ec(tc.tile_pool(name="ctxps", bufs=2, space="PSUM"))
            # ---- persistent staging ----
            def load_w(dram_t):
                t = singles.tile([P, 4, H], bf, tag=f"w_{dram_t.name}")
                nc.sync.dma_start(out=t, in_=dram_t.rearrange("(fc p) j -> p fc j", p=P))
                return t

            w1s_t = load_w(w1s_d)
            w1b_t = load_w(w1b_d)
            wq_t = load_w(wq_d)
            wk_t = load_w(wk_d)
            wv_t = load_w(wv_d)
            wo_t = load_w(wo_d)
            hcf_t = singles.tile([P, 4, BL], bf, tag="hcf")
            nc.sync.dma_start(out=hcf_t, in_=hcf_d.rearrange("(fc p) c -> p fc c", p=P))
            mb_t = singles.tile([P, CT, N], f32, tag="mb")
            nc.sync.dma_start(out=mb_t, in_=mb_d.rearrange("ct c n -> c ct n"))

            identb = singles.tile([P, P], bf, tag="identb")
            make_identity(nc, identb)
            identf = singles.tile([P, P], f32, tag="identf")
            make_identity(nc, identf)

            qs_all = singles.tile([P, CT, H], f32, tag="qs_all")
            sumt = singles.tile([P, CT, N], f32, tag="sumt")
            sumsq = singles.tile([P, CT, N], f32, tag="sumsq")
            rs_all = singles.tile([P, CT, N], f32, tag="rs_all")
            nmurs = singles.tile([P, CT, N], f32, tag="nmurs")
            scores = singles.tile([P, CT, NHEAD, N], f32, tag="scores")
            attn = singles.tile([P, CT, NHEAD, N], f32, tag="attn")

            t_spill = dram.tile([CT, N, P, H], bf, tag="t_spill")
            h_spill = dram.tile([CT, N, P, 4, P], bf, tag="h_spill")

            # ---- P0 + PI ----
            for ct in range(CT):
                zps = mmps.tile([P, H], f32, tag="mm")
                for fc in range(4):
                    nc.tensor.matmul(zps, hcf_t[:, fc, ct * P:(ct + 1) * P],
                                     w1b_t[:, fc], start=fc == 0, stop=fc == 3)
                zb = zbp.tile([P, H], f32, tag="zb")
                nc.scalar.copy(out=zb, in_=zps)
                qps = mmps.tile([P, H], f32, tag="mm")
                for fc in range(4):
                    nc.tensor.matmul(qps, hcf_t[:, fc, ct * P:(ct + 1) * P],
                                     wq_t[:, fc], start=fc == 0, stop=fc == 3)
                nc.scalar.copy(out=qs_all[:, ct], in_=qps)

                for half in range(N // 2):
                    hin = hnin.tile([P, 4, 2, P], bf, tag="hnin")
                    nc.sync.dma_start(
                        out=hin, in_=hnf_re[ct, :, :, 2 * half:2 * half + 2, :])
                    for k in range(2):
                        n = 2 * half + k
                        pre = mmps.tile([P, H], f32, tag="mm")
                        for fc in range(4):
                            nc.tensor.matmul(pre, hin[:, fc, k], w1s_t[:, fc],
                                             start=fc == 0, stop=fc == 3)
                        tsb = tp.tile([P, H], bf, tag="t")
                        nc.vector.scalar_tensor_tensor(
                            out=tsb, in0=pre, scalar=0.0, in1=zb,
                            op0=Alu.add, op1=Alu.subtract,
                            accum_out=sumt[:, ct, n:n + 1])
                        sq = sqs.tile([P, H], bf, tag="sq")
                        nc.scalar.activation(out=sq, in_=tsb, func=Act.Square,
                                             accum_out=sumsq[:, ct, n:n + 1])
                        nc.sync.dma_start(out=t_spill[ct, n], in_=tsb)

            # ---- PII: rs = 1/sqrt(var+eps), nmurs = -mu*rs (batched) ----
            F = CT * N
            sumt_f = sumt.rearrange("c ct n -> c (ct n)")
            sumsq_f = sumsq.rearrange("c ct n -> c (ct n)")
            rs_f = rs_all.rearrange("c ct n -> c (ct n)")
            nmurs_f = nmurs.rearrange("c ct n -> c (ct n)")
            musq = singles.tile([P, F], f32, tag="musq")
            var_t = singles.tile([P, F], f32, tag="var")
            sd_t = singles.tile([P, F], f32, tag="sd")
            nc.vector.tensor_mul(musq, sumt_f, sumt_f)
            nc.vector.tensor_scalar_mul(musq, musq, 1.0 / H)
            nc.vector.tensor_sub(var_t, sumsq_f, musq)
            # sd = sqrt(var/H + eps)
            eps_t = singles.tile([P, 1], f32, tag="eps")
            nc.vector.memset(eps_t, LN_EPS)
            nc.scalar.activation(out=sd_t, in_=var_t, func=Act.Sqrt,
                                 scale=1.0 / H, bias=eps_t)
            nc.vector.reciprocal(rs_f, sd_t)
            nc.vector.tensor_mul(nmurs_f, sumt_f, rs_f)
            nc.vector.tensor_scalar_mul(nmurs_f, nmurs_f, -1.0 / H)

            # ---- PIII: gelu, transpose, K, scores ----
            for ct in range(CT):
                for n in range(N):
                    t_rd = trd.tile([P, H], bf, tag="trd")
                    nc.sync.dma_start(out=t_rd, in_=t_spill[ct, n])
                    hn = hnp.tile([P, H], bf, tag="hn")
                    nc.scalar.activation(out=hn, in_=t_rd, func=Act.Gelu,
                                         scale=rs_all[:, ct, n:n + 1],
                                         bias=nmurs[:, ct, n:n + 1])
                    tps = trps.tile([P, 4, P], bf, tag="tr")
                    for fc in range(4):
                        nc.tensor.transpose(tps[:, fc], hn[:, fc * P:(fc + 1) * P],
                                            identb)
                    hf_t = hfs.tile([P, 4, P], bf, tag="hfs")
                    nc.scalar.copy(out=hf_t, in_=tps)
                    nc.sync.dma_start(out=h_spill[ct, n], in_=hf_t)
                    kps = mmps.tile([P, H], f32, tag="mm")
                    for fc in range(4):
                        nc.tensor.matmul(kps, hf_t[:, fc], wk_t[:, fc],
                                         start=fc == 0, stop=fc == 3)
                    pkt = pkp.tile([P, H], f32, tag="pk")
                    nc.vector.tensor_mul(pkt, kps, qs_all[:, ct])
                    nc.vector.reduce_sum(
                        out=scores[:, ct, :, n],
                        in_=pkt.rearrange("c (h d) -> c h d", h=NHEAD), axis=X)

            # ---- PIV: batched masked softmax ----
            sc_f = scores.rearrange("c ct h n -> c (ct h n)")
            sc2 = singles.tile([P, CT, NHEAD, N], f32, tag="sc2")
            sc3 = singles.tile([P, CT, NHEAD, N], f32, tag="sc3")
            e_all = singles.tile([P, CT, NHEAD, N], f32, tag="e_all")
            smax = singles.tile([P, CT, NHEAD], f32, tag="smax")
            ssum = singles.tile([P, CT, NHEAD], f32, tag="ssum")
            nc.vector.tensor_add(
                sc2, scores, mb_t[:, :, None, :].to_broadcast((P, CT, NHEAD, N)))
            nc.vector.reduce_max(out=smax, in_=sc2, axis=X)
            nc.vector.tensor_sub(
                sc3, sc2, smax[:, :, :, None].to_broadcast((P, CT, NHEAD, N)))
            nc.scalar.activation(out=e_all, in_=sc3, func=Act.Exp)
            nc.vector.reduce_sum(out=ssum, in_=e_all, axis=X)
            nc.vector.reciprocal(ssum, ssum)
            nc.vector.tensor_mul(
                attn, e_all, ssum[:, :, :, None].to_broadcast((P, CT, NHEAD, N)))

            # ---- PV: V, weighted context, Wo ----
            for ct in range(CT):
                ctx = ctxps.tile([P, H], f32, tag="ctx")
                for n in range(N):
                    hf_r = hfr.tile([P, 4, P], bf, tag="hfr")
                    nc.sync.dma_start(out=hf_r, in_=h_spill[ct, n])
                    ax = axp.tile([P, NHEAD, DH], f32, tag="ax")
                    nc.gpsimd.tensor_copy(
                        out=ax,
                        in_=attn[:, ct, :, n:n + 1].to_broadcast((P, NHEAD, DH)))
                    vps = mmps.tile([P, H], f32, tag="mm")
                    for fc in range(4):
                        nc.tensor.matmul(vps, hf_r[:, fc], wv_t[:, fc],
                                         start=fc == 0, stop=fc == 3)
                    pvt = pvp.tile([P, H], f32, tag="pv")
                    nc.vector.tensor_mul(pvt, vps,
                                         ax.rearrange("c h d -> c (h d)"))
                    nc.tensor.matmul(ctx, identf, pvt,
                                     start=n == 0, stop=n == N - 1)
                cs = tail.tile([P, H], bf, tag="cs")
                nc.scalar.copy(out=cs, in_=ctx)
                cts = trps.tile([P, 4, P], bf, tag="tr")
                for fc in range(4):
                    nc.tensor.transpose(cts[:, fc], cs[:, fc * P:(fc + 1) * P],
                                        identb)
                ctf = tail.tile([P, 4, P], bf, tag="ctf")
                nc.scalar.copy(out=ctf, in_=cts)
                ops = mmps.tile([P, H], f32, tag="mm")
                for fc in range(4):
                    nc.tensor.matmul(ops, ctf[:, fc], wo_t[:, fc],
                                     start=fc == 0, stop=fc == 3)
                ot = tail.tile([P, H], f32, tag="ot")
                nc.scalar.copy(out=ot, in_=ops)
                nc.sync.dma_start(out=out_d[ct * P:(ct + 1) * P, :], in_=ot)

    nc.finalize()
    return nc


def _get_nc():
    if "nc" not in _CACHE:
        _CACHE["nc"] = _build_nc()
    return _CACHE["nc"]


def _pack_inputs(h_center, h_neighbors, neighbor_mask, W1, Wq, Wk, Wv, Wo):
    hn = np.asarray(h_neighbors, np.float32)
    hc = np.asarray(h_center, np.float32)
    mask = np.asarray(neighbor_mask)
    W1 = np.asarray(W1, np.float32)
    w1s = (W1[:H] + W1[H:]).astype(bf16)
    w1b = W1[H:].astype(bf16)
    wq = (np.asarray(Wq, np.float32) / np.sqrt(DH)).astype(bf16)
    wk = np.asarray(Wk, bf16)
    wv = np.asarray(Wv, bf16)
    wo = np.asarray(Wo, bf16)

    # [m, ct, f, n, c] feature-major neighbor tensor
    hnf = np.ascontiguousarray(
        hn.reshape(M, CT, P, N, H).transpose(0, 1, 4, 3, 2)).astype(bf16)
    hcf = np.ascontiguousarray(
        hc.reshape(M, BL, H).transpose(0, 2, 1)).astype(bf16)
    mb = np.where(mask, 0.0, NEG).astype(np.float32).reshape(M, CT, P, N)

    in_maps = []
    for m in range(M):
        in_maps.append({
            "hnf": np.ascontiguousarray(hnf[m]),
            "hcf": np.ascontiguousarray(hcf[m]),
            "mb": np.ascontiguousarray(mb[m]),
            "w1s": w1s, "w1b": w1b, "wq": wq, "wk": wk, "wv": wv, "wo": wo,
        })
    return in_maps


def _fast_path_ok(b1, ln_g, ln_b, bq, bk, bv, bo):
    return (np.all(np.asarray(b1) == 0) and np.all(np.asarray(ln_g) == 1)
            and np.all(np.asarray(ln_b) == 0) and np.all(np.asarray(bq) == 0)
            and np.all(np.asarray(bv) == 0) and np.all(np.asarray(bo) == 0))


def _np_fallback(h_center, h_neighbors, W1, b1, ln_g, ln_b, Wq, bq, Wk, bk,
                 Wv, bv, Wo, bo, neighbor_mask):
    from scipy.special import erf

    hc = np.asarray(h_center, np.float32)
    hn = np.asarray(h_neighbors, np.float32)
    diff = hn - hc[:, None, :]
    comb = np.concatenate([hn, diff], -1)
    pre = comb @ W1 + b1
    mu = pre.mean(-1, keepdims=True)
    var = ((pre - mu) ** 2).mean(-1, keepdims=True)
    x = (pre - mu) / np.sqrt(var + LN_EPS) * ln_g + ln_b
    hnp_ = 0.5 * x * (1 + erf(x / np.sqrt(2)))
    Q = (hc @ Wq + bq).reshape(B, NHEAD, DH)
    K = (hnp_ @ Wk + bk).reshape(B, N, NHEAD, DH)
    V = (hnp_ @ Wv + bv).reshape(B, N, NHEAD, DH)
    sc = np.einsum("bhd,bnhd->bhn", Q, K) / np.sqrt(DH)
    sc = np.where(neighbor_mask[:, None, :], sc, -np.inf)
    sc = sc - sc.max(-1, keepdims=True)
    e = np.exp(sc)
    attn = e / e.sum(-1, keepdims=True)
    ctx = np.einsum("bhn,bnhd->bhd", attn, V).reshape(B, H)
    return (ctx @ Wo + bo).astype(np.float32)


def run_spmd(in_maps, **kwargs):
    from concourse.bass_utils import run_bass_kernel_spmd

    return run_bass_kernel_spmd(_get_nc(), in_maps, core_ids=list(range(M)),
                                **kwargs)


def kernel(h_center, h_neighbors, W1, b1, ln_g, ln_b, Wq, bq, Wk, bk, Wv, bv,
           Wo, bo, neighbor_mask):
    if not _fast_path_ok(b1, ln_g, ln_b, bq, bk, bv, bo):
        return _np_fallback(h_center, h_neighbors, W1, b1, ln_g, ln_b, Wq, bq,
                            Wk, bk, Wv, bv, Wo, bo, neighbor_mask)
    in_maps = _pack_inputs(h_center, h_neighbors, neighbor_mask, W1, Wq, Wk,
                           Wv, Wo)
    res = run_spmd(in_maps)
    return np.concatenate([r["out"] for r in res.results], axis=0)


# revision 9
# speedup vs baseline: 1.0009x; 1.0009x over previous
"""DifferenceAwareAggregator — Bass/Tile kernel, data-parallel on 8 NeuronCores.

Sharding: batch dim (B=8192) split across 8 cores (1024 centers each);
small projection weights replicated. Each core runs a 5-phase Tile kernel:

  PI   pre = Hn @ W1s - Hc @ W1b   (feature-major inputs pre-packed on host,
       so no on-chip input transposes); LN stats fused into the PSUM
       evacuation (scalar_tensor_tensor accum_out) + ACT Square accum_out;
       t spilled to HBM in bf16.
  PII  batched rsqrt of the LN variances (one sqrt-set excursion).
  PIII hn = Gelu(t*rs - mu*rs) fused on ScalarE; PE-transpose hn; K = hn@Wk;
       scores = head-reduce(Q * K).
  PIV  batched masked softmax over all 8 center-tiles (one exp-set excursion).
  PV   V = hn@Wv; prod = V * attn; ctx accumulated over neighbors in PSUM
       via identity-matmul; out = ctx @ Wo.

Algebra: concat([h_n, h_n - h_c]) @ W1 == h_n @ (W1top+W1bot) - h_c @ W1bot.
bk drops out of softmax (per-(b,h) constant shift). 1/sqrt(d) folded into Wq.
"""

import sys

import numpy as np
import ml_dtypes

_TRN = "/opt/trn_rl_repo"
if _TRN not in sys.path:
    sys.path.insert(0, _TRN)

bf16 = ml_dtypes.bfloat16

M = 8          # cores
B = 8192
N = 32         # neighbors
H = 512
BL = B // M    # centers per core
P = 128        # partitions
CT = BL // P   # center tiles per core
NHEAD = 8
DH = H // NHEAD
NEG = -30000.0
LN_EPS = 1e-5

_CACHE: dict = {}


def _build_nc():
    import concourse.mybir as mybir
    import concourse.tile as tile
    from concourse import bacc
    from concourse.masks import make_identity

    f32 = mybir.dt.float32
    bf = mybir.dt.bfloat16
    Alu = mybir.AluOpType
    Act = mybir.ActivationFunctionType
    X = mybir.AxisListType.X

    nc = bacc.Bacc()
    hnf_d = nc.dram_tensor("hnf", [CT, H, N, P], bf, kind="ExternalInput")
    hcf_d = nc.dram_tensor("hcf", [H, BL], bf, kind="ExternalInput")
    mb_d = nc.dram_tensor("mb", [CT, P, N], f32, kind="ExternalInput")
    w1s_d = nc.dram_tensor("w1s", [H, H], bf, kind="ExternalInput")
    w1b_d = nc.dram_tensor("w1b", [H, H], bf, kind="ExternalInput")
    wq_d = nc.dram_tensor("wq", [H, H], bf, kind="ExternalInput")
    wk_d = nc.dram_tensor("wk", [H, H], bf, kind="ExternalInput")
    wv_d = nc.dram_tensor("wv", [H, H], bf, kind="ExternalInput")
    wo_d = nc.dram_tensor("wo", [H, H], bf, kind="ExternalInput")
    out_d = nc.dram_tensor("out", [BL, H], f32, kind="ExternalOutput")

    hnf_re = hnf_d.rearrange("ct (fc p) n c -> ct p fc n c", p=P)

    from contextlib import ExitStack

    with tile.TileContext(nc) as tc:
        with ExitStack() as stack:
            ec = stack.enter_context
            singles = ec(tc.tile_pool(name="singles", bufs=1))
            dram = ec(tc.tile_pool(name="dram", bufs=1, space="DRAM"))
            zbp = ec(tc.tile_pool(name="zb", bufs=2))
            hnin = ec(tc.tile_pool(name="hnin", bufs=5))
            tp = ec(tc.tile_pool(name="tp", bufs=3))
            sqs = ec(tc.tile_pool(name="sqs", bufs=2))
            trd = ec(tc.tile_pool(name="trd", bufs=4))
            hnp = ec(tc.tile_pool(name="hnp", bufs=3))
            hfs = ec(tc.tile_pool(name="hfs", bufs=3))
            pkp = ec(tc.tile_pool(name="pk", bufs=3))
            hfr = ec(tc.tile_pool(name="hfr", bufs=4))
            axp = ec(tc.tile_pool(name="axp", bufs=3))
            pvp = ec(tc.tile_pool(name="pv", bufs=3))
            tail = ec(tc.tile_pool(name="tail", bufs=2))
            mmps = ec(tc.tile_pool(name="mmps", bufs=3, space="PSUM"))
            trps = ec(tc.tile_pool(name="trps", bufs=2, space="PSUM"))
            ctxps = ec(tc.tile_pool(name="ctxps", bufs=2, space="PSUM"))
            # ---- persistent staging ----
            def load_w(dram_t):
                t = singles.tile([P, 4, H], bf, tag=f"w_{dram_t.name}")
                nc.sync.dma_start(out=t, in_=dram_t.rearrange("(fc p) j -> p fc j", p=P))
                return t

            hcf_t = singles.tile([P, 4, BL], bf, tag="hcf")
            nc.sync.dma_start(out=hcf_t, in_=hcf_d.rearrange("(fc p) c -> p fc c", p=P))
            w1b_t = load_w(w1b_d)
            w1s_t = load_w(w1s_d)
            wq_t = load_w(wq_d)
            wk_t = load_w(wk_d)
            wv_t = load_w(wv_d)
            wo_t = load_w(wo_d)
            mb_t = singles.tile([P, CT, N], f32, tag="mb")
            nc.sync.dma_start(out=mb_t, in_=mb_d.rearrange("ct c n -> c ct n"))

            identb = singles.tile([P, P], bf, tag="identb")
            make_identity(nc, identb)

            qs_all = singles.tile([P, CT, H], f32, tag="qs_all")
            sumt = singles.tile([P, CT, N], f32, tag="sumt")
            sumsq = singles.tile([P, CT, N], f32, tag="sumsq")
            rs_all = singles.tile([P, CT, N], f32, tag="rs_all")
            nmurs = singles.tile([P, CT, N], f32, tag="nmurs")
            scores = singles.tile([P, CT, NHEAD, N], f32, tag="scores")
            attn = singles.tile([P, CT, NHEAD, N], f32, tag="attn")

            t_spill = dram.tile([CT, N // 4, P, 4, H], bf, tag="t_spill")
            h_spill = dram.tile([CT, N // 4, P, 4, 4, P], bf, tag="h_spill")

            # ---- P0 + PI ----
            for ct in range(CT):
                zps = mmps.tile([P, H], f32, tag="mm")
                for fc in range(4):
                    nc.tensor.matmul(zps, hcf_t[:, fc, ct * P:(ct + 1) * P],
                                     w1b_t[:, fc], start=fc == 0, stop=fc == 3)
                zb = zbp.tile([P, H], f32, tag="zb")
                nc.scalar.copy(out=zb, in_=zps)
                qps = mmps.tile([P, H], f32, tag="mm")
                for fc in range(4):
                    nc.tensor.matmul(qps, hcf_t[:, fc, ct * P:(ct + 1) * P],
                                     wq_t[:, fc], start=fc == 0, stop=fc == 3)
                nc.scalar.copy(out=qs_all[:, ct], in_=qps)

                for grp in range(N // 4):
                    hin = hnin.tile([P, 4, 4, P], bf, tag="hnin")
                    nc.sync.dma_start(
                        out=hin, in_=hnf_re[ct, :, :, 4 * grp:4 * grp + 4, :])
                    tsb = tp.tile([P, 4, H], bf, tag="t")
                    for k in range(4):
                        n = 4 * grp + k
                        pre = mmps.tile([P, H], f32, tag="mm")
                        for fc in range(4):
                            nc.tensor.matmul(pre, hin[:, fc, k], w1s_t[:, fc],
                                             start=fc == 0, stop=fc == 3)
                        nc.vector.scalar_tensor_tensor(
                            out=tsb[:, k], in0=pre, scalar=0.0, in1=zb,
                            op0=Alu.add, op1=Alu.subtract,
                            accum_out=sumt[:, ct, n:n + 1])
                        sq = sqs.tile([P, H], bf, tag="sq")
                        nc.scalar.activation(out=sq, in_=tsb[:, k],
                                             func=Act.Square,
                                             accum_out=sumsq[:, ct, n:n + 1])
                    nc.gpsimd.dma_start(out=t_spill[ct, grp], in_=tsb)

            # ---- PII: rs = 1/sqrt(var+eps), nmurs = -mu*rs (batched) ----
            F = CT * N
            sumt_f = sumt.rearrange("c ct n -> c (ct n)")
            sumsq_f = sumsq.rearrange("c ct n -> c (ct n)")
            rs_f = rs_all.rearrange("c ct n -> c (ct n)")
            nmurs_f = nmurs.rearrange("c ct n -> c (ct n)")
            musq = singles.tile([P, F], f32, tag="musq")
            var_t = singles.tile([P, F], f32, tag="var")
            sd_t = singles.tile([P, F], f32, tag="sd")
            nc.vector.tensor_mul(musq, sumt_f, sumt_f)
            nc.vector.tensor_scalar_mul(musq, musq, 1.0 / H)
            nc.vector.tensor_sub(var_t, sumsq_f, musq)
            # sd = sqrt(var/H + eps)
            eps_t = singles.tile([P, 1], f32, tag="eps")
            nc.vector.memset(eps_t, LN_EPS)
            nc.scalar.activation(out=sd_t, in_=var_t, func=Act.Sqrt,
                                 scale=1.0 / H, bias=eps_t)
            nc.vector.reciprocal(rs_f, sd_t)
            nc.vector.tensor_mul(nmurs_f, sumt_f, rs_f)
            nc.vector.tensor_scalar_mul(nmurs_f, nmurs_f, -1.0 / H)

            # ---- PIII: gelu, transpose, K, scores ----
            for ct in range(CT):
                for grp in range(N // 4):
                    t_rd = trd.tile([P, 4, H], bf, tag="trd")
                    nc.sync.dma_start(out=t_rd, in_=t_spill[ct, grp])
                    hf_t = hfs.tile([P, 4, 4, P], bf, tag="hfs")
                    for k in range(4):
                        n = 4 * grp + k
                        hn = hnp.tile([P, H], bf, tag="hn")
                        nc.scalar.activation(out=hn, in_=t_rd[:, k],
                                             func=Act.Gelu,
                                             scale=rs_all[:, ct, n:n + 1],
                                             bias=nmurs[:, ct, n:n + 1])
                        tps = trps.tile([P, 4, P], bf, tag="tr")
                        for fc in range(4):
                            nc.tensor.transpose(tps[:, fc],
                                                hn[:, fc * P:(fc + 1) * P],
                                                identb)
                        nc.scalar.copy(out=hf_t[:, k], in_=tps)
                        kps = mmps.tile([P, H], f32, tag="mm")
                        for fc in range(4):
                            nc.tensor.matmul(kps, hf_t[:, k, fc], wk_t[:, fc],
                                             start=fc == 0, stop=fc == 3)
                        pkt = pkp.tile([P, H], f32, tag="pk")
                        nc.vector.tensor_mul(pkt, kps, qs_all[:, ct])
                        nc.vector.reduce_sum(
                            out=scores[:, ct, :, n],
                            in_=pkt.rearrange("c (h d) -> c h d", h=NHEAD),
                            axis=X)
                    nc.gpsimd.dma_start(out=h_spill[ct, grp], in_=hf_t)

            # ---- PIV: batched masked softmax ----
            sc_f = scores.rearrange("c ct h n -> c (ct h n)")
            sc2 = singles.tile([P, CT, NHEAD, N], f32, tag="sc2")
            e_all = singles.tile([P, CT, NHEAD, N], f32, tag="e_all")
            ssum = singles.tile([P, CT, NHEAD], f32, tag="ssum")
            nc.vector.tensor_add(
                sc2, scores, mb_t[:, :, None, :].to_broadcast((P, CT, NHEAD, N)))
            nc.scalar.activation(out=e_all, in_=sc2, func=Act.Exp)
            nc.vector.reduce_sum(out=ssum, in_=e_all, axis=X)
            nc.vector.reciprocal(ssum, ssum)
            nc.vector.tensor_mul(
                attn, e_all, ssum[:, :, :, None].to_broadcast((P, CT, NHEAD, N)))

            # ---- PV: V, weighted context, Wo ----
            for ct in range(CT):
                ctx = ctxps.tile([P, H], f32, tag="ctx")
                for grp in range(N // 4):
                    hf_r = hfr.tile([P, 4, 4, P], bf, tag="hfr")
                    nc.sync.dma_start(out=hf_r, in_=h_spill[ct, grp])
                    for k in range(4):
                        n = 4 * grp + k
                        ax = axp.tile([P, NHEAD, DH], f32, tag="ax")
                        nc.gpsimd.tensor_copy(
                            out=ax,
                            in_=attn[:, ct, :, n:n + 1].to_broadcast(
                                (P, NHEAD, DH)))
                        vps = mmps.tile([P, H], f32, tag="mm")
                        for fc in range(4):
                            nc.tensor.matmul(vps, hf_r[:, k, fc], wv_t[:, fc],
                                             start=fc == 0, stop=fc == 3)
                        pvt = pvp.tile([P, H], bf, tag="pv")
                        nc.vector.tensor_mul(pvt, vps,
                                             ax.rearrange("c h d -> c (h d)"))
                        nc.tensor.matmul(ctx, identb, pvt,
                                         start=n == 0, stop=n == N - 1)
                cs = tail.tile([P, H], bf, tag="cs")
                nc.scalar.copy(out=cs, in_=ctx)
                cts = trps.tile([P, 4, P], bf, tag="tr")
                for fc in range(4):
                    nc.tensor.transpose(cts[:, fc], cs[:, fc * P:(fc + 1) * P],
                                        identb)
                ctf = tail.tile([P, 4, P], bf, tag="ctf")
                nc.scalar.copy(out=ctf, in_=cts)
                ops = mmps.tile([P, H], f32, tag="mm")
                for fc in range(4):
                    nc.tensor.matmul(ops, ctf[:, fc], wo_t[:, fc],
                                     start=fc == 0, stop=fc == 3)
                ot = tail.tile([P, H], f32, tag="ot")
                nc.scalar.copy(out=ot, in_=ops)
                nc.sync.dma_start(out=out_d[ct * P:(ct + 1) * P, :], in_=ot)

    nc.finalize()
    return nc


def _get_nc():
    if "nc" not in _CACHE:
        _CACHE["nc"] = _build_nc()
    return _CACHE["nc"]


def _pack_inputs(h_center, h_neighbors, neighbor_mask, W1, Wq, Wk, Wv, Wo):
    hn = np.asarray(h_neighbors, np.float32)
    hc = np.asarray(h_center, np.float32)
    mask = np.asarray(neighbor_mask)
    W1 = np.asarray(W1, np.float32)
    w1s = (W1[:H] + W1[H:]).astype(bf16)
    w1b = W1[H:].astype(bf16)
    wq = (np.asarray(Wq, np.float32) / np.sqrt(DH)).astype(bf16)
    wk = np.asarray(Wk, bf16)
    wv = np.asarray(Wv, bf16)
    wo = np.asarray(Wo, bf16)

    # [m, ct, f, n, c] feature-major neighbor tensor
    hnf = np.ascontiguousarray(
        hn.reshape(M, CT, P, N, H).transpose(0, 1, 4, 3, 2)).astype(bf16)
    hcf = np.ascontiguousarray(
        hc.reshape(M, BL, H).transpose(0, 2, 1)).astype(bf16)
    mb = np.where(mask, 0.0, NEG).astype(np.float32).reshape(M, CT, P, N)

    in_maps = []
    for m in range(M):
        in_maps.append({
            "hnf": np.ascontiguousarray(hnf[m]),
            "hcf": np.ascontiguousarray(hcf[m]),
            "mb": np.ascontiguousarray(mb[m]),
            "w1s": w1s, "w1b": w1b, "wq": wq, "wk": wk, "wv": wv, "wo": wo,
        })
    return in_maps


def _fast_path_ok(b1, ln_g, ln_b, bq, bk, bv, bo):
    return (np.all(np.asarray(b1) == 0) and np.all(np.asarray(ln_g) == 1)
            and np.all(np.asarray(ln_b) == 0) and np.all(np.asarray(bq) == 0)
            and np.all(np.asarray(bv) == 0) and np.all(np.asarray(bo) == 0))


def _np_fallback(h_center, h_neighbors, W1, b1, ln_g, ln_b, Wq, bq, Wk, bk,
                 Wv, bv, Wo, bo, neighbor_mask):
    from scipy.special import erf

    hc = np.asarray(h_center, np.float32)
    hn = np.asarray(h_neighbors, np.float32)
    diff = hn - hc[:, None, :]
    comb = np.concatenate([hn, diff], -1)
    pre = comb @ W1 + b1
    mu = pre.mean(-1, keepdims=True)
    var = ((pre - mu) ** 2).mean(-1, keepdims=True)
    x = (pre - mu) / np.sqrt(var + LN_EPS) * ln_g + ln_b
    hnp_ = 0.5 * x * (1 + erf(x / np.sqrt(2)))
    Q = (hc @ Wq + bq).reshape(B, NHEAD, DH)
    K = (hnp_ @ Wk + bk).reshape(B, N, NHEAD, DH)
    V = (hnp_ @ Wv + bv).reshape(B, N, NHEAD, DH)
    sc = np.einsum("bhd,bnhd->bhn", Q, K) / np.sqrt(DH)
    sc = np.where(neighbor_mask[:, None, :], sc, -np.inf)
    sc = sc - sc.max(-1, keepdims=True)
    e = np.exp(sc)
    attn = e / e.sum(-1, keepdims=True)
    ctx = np.einsum("bhn,bnhd->bhd", attn, V).reshape(B, H)
    return (ctx @ Wo + bo).astype(np.float32)


def run_spmd(in_maps, **kwargs):
    from concourse.bass_utils import run_bass_kernel_spmd

    return run_bass_kernel_spmd(_get_nc(), in_maps, core_ids=list(range(M)),
                                **kwargs)


def kernel(h_center, h_neighbors, W1, b1, ln_g, ln_b, Wq, bq, Wk, bk, Wv, bv,
           Wo, bo, neighbor_mask):
    if not _fast_path_ok(b1, ln_g, ln_b, bq, bk, bv, bo):
        return _np_fallback(h_center, h_neighbors, W1, b1, ln_g, ln_b, Wq, bq,
                            Wk, bk, Wv, bv, Wo, bo, neighbor_mask)
    in_maps = _pack_inputs(h_center, h_neighbors, neighbor_mask, W1, Wq, Wk,
                           Wv, Wo)
    res = run_spmd(in_maps)
    return np.concatenate([r["out"] for r in res.results], axis=0)


# revision 13
# speedup vs baseline: 48.8541x; 48.8081x over previous
"""DifferenceAwareAggregator — Bass/Tile kernel, data-parallel on 8 NeuronCores.

Sharding: batch dim (B=8192) split across 8 cores (1024 centers each);
small projection weights replicated. Each core runs a 5-phase Tile kernel:

  PI   pre = Hn @ W1s - Hc @ W1b   (feature-major inputs pre-packed on host,
       so no on-chip input transposes); LN stats fused into the PSUM
       evacuation (scalar_tensor_tensor accum_out) + ACT Square accum_out;
       t spilled to HBM in bf16.
  PII  batched rsqrt of the LN variances (one sqrt-set excursion).
  PIII hn = Gelu(t*rs - mu*rs) fused on ScalarE; PE-transpose hn; K = hn@Wk;
       scores = head-reduce(Q * K).
  PIV  batched masked softmax over all 8 center-tiles (one exp-set excursion).
  PV   V = hn@Wv; prod = V * attn; ctx accumulated over neighbors in PSUM
       via identity-matmul; out = ctx @ Wo.

Algebra: concat([h_n, h_n - h_c]) @ W1 == h_n @ (W1top+W1bot) - h_c @ W1bot.
bk drops out of softmax (per-(b,h) constant shift). 1/sqrt(d) folded into Wq.
"""

import sys

import numpy as np
import ml_dtypes

_TRN = "/opt/trn_rl_repo"
if _TRN not in sys.path:
    sys.path.insert(0, _TRN)

bf16 = ml_dtypes.bfloat16

M = 8          # cores
B = 8192
N = 32         # neighbors
H = 512
BL = B // M    # centers per core
P = 128        # partitions
CT = BL // P   # center tiles per core
NHEAD = 8
DH = H // NHEAD
NEG = -30000.0
LN_EPS = 1e-5

_CACHE: dict = {}


def _build_nc():
    import concourse.mybir as mybir
    import concourse.tile as tile
    from concourse import bacc
    from concourse.masks import make_identity

    f32 = mybir.dt.float32
    bf = mybir.dt.bfloat16
    Alu = mybir.AluOpType
    Act = mybir.ActivationFunctionType
    X = mybir.AxisListType.X

    nc = bacc.Bacc()
    hnf_d = nc.dram_tensor("hnf", [CT, H, N, P], bf, kind="ExternalInput")
    hcf_d = nc.dram_tensor("hcf", [H, BL], bf, kind="ExternalInput")
    mb_d = nc.dram_tensor("mb", [CT, P, N], f32, kind="ExternalInput")
    w1s_d = nc.dram_tensor("w1s", [H, H], bf, kind="ExternalInput")
    w1b_d = nc.dram_tensor("w1b", [H, H], bf, kind="ExternalInput")
    wq_d = nc.dram_tensor("wq", [H, H], bf, kind="ExternalInput")
    wk_d = nc.dram_tensor("wk", [H, H], bf, kind="ExternalInput")
    wv_d = nc.dram_tensor("wv", [H, H], bf, kind="ExternalInput")
    wo_d = nc.dram_tensor("wo", [H, H], bf, kind="ExternalInput")
    out_d = nc.dram_tensor("out", [BL, H], bf, kind="ExternalOutput")

    hnf_re = hnf_d.rearrange("ct (fc p) n c -> ct p fc n c", p=P)

    from contextlib import ExitStack

    with tile.TileContext(nc) as tc:
        with ExitStack() as stack:
            ec = stack.enter_context
            singles = ec(tc.tile_pool(name="singles", bufs=1))
            dram = ec(tc.tile_pool(name="dram", bufs=1, space="DRAM"))
            zbp = ec(tc.tile_pool(name="zb", bufs=2))
            hnin = ec(tc.tile_pool(name="hnin", bufs=5))
            tp = ec(tc.tile_pool(name="tp", bufs=10))
            sqs = ec(tc.tile_pool(name="sqs", bufs=2))
            trd = ec(tc.tile_pool(name="trd", bufs=4))
            hnp = ec(tc.tile_pool(name="hnp", bufs=3))
            hfs = ec(tc.tile_pool(name="hfs", bufs=3))
            pkp = ec(tc.tile_pool(name="pk", bufs=3))
            hfr = ec(tc.tile_pool(name="hfr", bufs=4))
            axp = ec(tc.tile_pool(name="axp", bufs=3))
            pvp = ec(tc.tile_pool(name="pv", bufs=3))
            tail = ec(tc.tile_pool(name="tail", bufs=2))
            mmps = ec(tc.tile_pool(name="mmps", bufs=3, space="PSUM"))
            trps = ec(tc.tile_pool(name="trps", bufs=2, space="PSUM"))
            ctxps = ec(tc.tile_pool(name="ctxps", bufs=2, space="PSUM"))
            # ---- persistent staging ----
            def load_w(dram_t):
                t = singles.tile([P, 4, H], bf, tag=f"w_{dram_t.name}")
                nc.sync.dma_start(out=t, in_=dram_t.rearrange("(fc p) j -> p fc j", p=P))
                return t

            hcf_t = singles.tile([P, 4, BL], bf, tag="hcf")
            nc.sync.dma_start(out=hcf_t, in_=hcf_d.rearrange("(fc p) c -> p fc c", p=P))
            w1b_t = load_w(w1b_d)
            w1s_t = load_w(w1s_d)
            wq_t = load_w(wq_d)
            wk_t = load_w(wk_d)
            wv_t = load_w(wv_d)
            wo_t = load_w(wo_d)
            mb_t = singles.tile([P, CT, N], f32, tag="mb")
            nc.sync.dma_start(out=mb_t, in_=mb_d.rearrange("ct c n -> c ct n"))

            identb = singles.tile([P, P], bf, tag="identb")
            make_identity(nc, identb)

            qs_all = singles.tile([P, CT, H], f32, tag="qs_all")
            sumt = singles.tile([P, CT, N], f32, tag="sumt")
            sumsq = singles.tile([P, CT, N], f32, tag="sumsq")
            rs_all = singles.tile([P, CT, N], f32, tag="rs_all")
            nmurs = singles.tile([P, CT, N], f32, tag="nmurs")
            scores = singles.tile([P, CT, NHEAD, N], f32, tag="scores")
            attn = singles.tile([P, CT, NHEAD, N], f32, tag="attn")

            t_spill = dram.tile([CT, N // 4, P, 4, H], bf, tag="t_spill")
            h_spill = dram.tile([CT, N // 4, P, 4, 4, P], bf, tag="h_spill")

            # ---- P0 + PI ----
            for ct in range(CT):
                zps = mmps.tile([P, H], f32, tag="mm")
                for fc in range(4):
                    nc.tensor.matmul(zps, hcf_t[:, fc, ct * P:(ct + 1) * P],
                                     w1b_t[:, fc], start=fc == 0, stop=fc == 3)
                zb = zbp.tile([P, H], f32, tag="zb")
                nc.scalar.copy(out=zb, in_=zps)
                qps = mmps.tile([P, H], f32, tag="mm")
                for fc in range(4):
                    nc.tensor.matmul(qps, hcf_t[:, fc, ct * P:(ct + 1) * P],
                                     wq_t[:, fc], start=fc == 0, stop=fc == 3)
                nc.scalar.copy(out=qs_all[:, ct], in_=qps)

                for grp in range(N // 4):
                    hin = hnin.tile([P, 4, 4, P], bf, tag="hnin")
                    nc.sync.dma_start(
                        out=hin, in_=hnf_re[ct, :, :, 4 * grp:4 * grp + 4, :])
                    tsb = tp.tile([P, 4, H], bf, tag="t")
                    for k in range(4):
                        n = 4 * grp + k
                        pre = mmps.tile([P, H], f32, tag="mm")
                        for fc in range(4):
                            nc.tensor.matmul(pre, hin[:, fc, k], w1s_t[:, fc],
                                             start=fc == 0, stop=fc == 3)
                        nc.vector.scalar_tensor_tensor(
                            out=tsb[:, k], in0=pre, scalar=0.0, in1=zb,
                            op0=Alu.add, op1=Alu.subtract,
                            accum_out=sumt[:, ct, n:n + 1])
                        sq = sqs.tile([P, H], bf, tag="sq")
                        nc.scalar.activation(out=sq, in_=tsb[:, k],
                                             func=Act.Square,
                                             accum_out=sumsq[:, ct, n:n + 1])
                    nc.gpsimd.dma_start(out=t_spill[ct, grp], in_=tsb)

            # ---- PII: rs = 1/sqrt(var+eps), nmurs = -mu*rs (batched) ----
            F = CT * N
            sumt_f = sumt.rearrange("c ct n -> c (ct n)")
            sumsq_f = sumsq.rearrange("c ct n -> c (ct n)")
            rs_f = rs_all.rearrange("c ct n -> c (ct n)")
            nmurs_f = nmurs.rearrange("c ct n -> c (ct n)")
            musq = singles.tile([P, F], f32, tag="musq")
            var_t = singles.tile([P, F], f32, tag="var")
            sd_t = singles.tile([P, F], f32, tag="sd")
            nc.vector.tensor_mul(musq, sumt_f, sumt_f)
            nc.vector.tensor_scalar_mul(musq, musq, 1.0 / H)
            nc.vector.tensor_sub(var_t, sumsq_f, musq)
            # sd = sqrt(var/H + eps)
            eps_t = singles.tile([P, 1], f32, tag="eps")
            nc.vector.memset(eps_t, LN_EPS)
            nc.scalar.activation(out=sd_t, in_=var_t, func=Act.Sqrt,
                                 scale=1.0 / H, bias=eps_t)
            nc.vector.reciprocal(rs_f, sd_t)
            nc.vector.tensor_mul(nmurs_f, sumt_f, rs_f)
            nc.vector.tensor_scalar_mul(nmurs_f, nmurs_f, -1.0 / H)

            # ---- PIII: gelu, transpose, K, scores ----
            for ct in range(CT):
                for grp in range(N // 4):
                    t_rd = trd.tile([P, 4, H], bf, tag="trd")
                    nc.sync.dma_start(out=t_rd, in_=t_spill[ct, grp])
                    hf_t = hfs.tile([P, 4, 4, P], bf, tag="hfs")
                    for k in range(4):
                        n = 4 * grp + k
                        hn = hnp.tile([P, H], bf, tag="hn")
                        nc.scalar.activation(out=hn, in_=t_rd[:, k],
                                             func=Act.Gelu,
                                             scale=rs_all[:, ct, n:n + 1],
                                             bias=nmurs[:, ct, n:n + 1])
                        tps = trps.tile([P, 4, P], bf, tag="tr")
                        for fc in range(4):
                            nc.tensor.transpose(tps[:, fc],
                                                hn[:, fc * P:(fc + 1) * P],
                                                identb)
                        nc.scalar.copy(out=hf_t[:, k], in_=tps)
                        kps = mmps.tile([P, H], f32, tag="mm")
                        for fc in range(4):
                            nc.tensor.matmul(kps, hf_t[:, k, fc], wk_t[:, fc],
                                             start=fc == 0, stop=fc == 3)
                        pkt = pkp.tile([P, H], f32, tag="pk")
                        nc.vector.tensor_mul(pkt, kps, qs_all[:, ct])
                        nc.vector.reduce_sum(
                            out=scores[:, ct, :, n],
                            in_=pkt.rearrange("c (h d) -> c h d", h=NHEAD),
                            axis=X)
                    nc.gpsimd.dma_start(out=h_spill[ct, grp], in_=hf_t)

            # ---- PIV: batched masked softmax ----
            sc_f = scores.rearrange("c ct h n -> c (ct h n)")
            sc2 = singles.tile([P, CT, NHEAD, N], f32, tag="sc2")
            e_all = singles.tile([P, CT, NHEAD, N], f32, tag="e_all")
            ssum = singles.tile([P, CT, NHEAD], f32, tag="ssum")
            nc.vector.tensor_add(
                sc2, scores, mb_t[:, :, None, :].to_broadcast((P, CT, NHEAD, N)))
            nc.scalar.activation(out=e_all, in_=sc2, func=Act.Exp)
            nc.vector.reduce_sum(out=ssum, in_=e_all, axis=X)
            nc.vector.reciprocal(ssum, ssum)
            nc.vector.tensor_mul(
                attn, e_all, ssum[:, :, :, None].to_broadcast((P, CT, NHEAD, N)))

            # ---- PV: V, weighted context, Wo ----
            for ct in range(CT):
                ctx = ctxps.tile([P, H], f32, tag="ctx")
                for grp in range(N // 4):
                    hf_r = hfr.tile([P, 4, 4, P], bf, tag="hfr")
                    nc.sync.dma_start(out=hf_r, in_=h_spill[ct, grp])
                    for k in range(4):
                        n = 4 * grp + k
                        ax = axp.tile([P, NHEAD, DH], f32, tag="ax")
                        nc.gpsimd.tensor_copy(
                            out=ax,
                            in_=attn[:, ct, :, n:n + 1].to_broadcast(
                                (P, NHEAD, DH)))
                        vps = mmps.tile([P, H], f32, tag="mm")
                        for fc in range(4):
                            nc.tensor.matmul(vps, hf_r[:, k, fc], wv_t[:, fc],
                                             start=fc == 0, stop=fc == 3)
                        pvt = pvp.tile([P, H], bf, tag="pv")
                        nc.vector.tensor_mul(pvt, vps,
                                             ax.rearrange("c h d -> c (h d)"))
                        nc.tensor.matmul(ctx, identb, pvt,
                                         start=n == 0, stop=n == N - 1)
                cs = tail.tile([P, H], bf, tag="cs")
                nc.scalar.copy(out=cs, in_=ctx)
                cts = trps.tile([P, 4, P], bf, tag="tr")
                for fc in range(4):
                    nc.tensor.transpose(cts[:, fc], cs[:, fc * P:(fc + 1) * P],
                                        identb)
                ctf = tail.tile([P, 4, P], bf, tag="ctf")
                nc.scalar.copy(out=ctf, in_=cts)
                ops = mmps.tile([P, H], f32, tag="mm")
                for fc in range(4):
                    nc.tensor.matmul(ops, ctf[:, fc], wo_t[:, fc],
                                     start=fc == 0, stop=fc == 3)
                ot = tail.tile([P, H], bf, tag="ot")
                nc.scalar.copy(out=ot, in_=ops)
                nc.sync.dma_start(out=out_d[ct * P:(ct + 1) * P, :], in_=ot)

    nc.finalize()
    return nc


def _get_nc():
    if "nc" not in _CACHE:
        _CACHE["nc"] = _build_nc()
    return _CACHE["nc"]


def _pack_inputs(h_center, h_neighbors, neighbor_mask, W1, Wq, Wk, Wv, Wo):
    hn = np.asarray(h_neighbors, np.float32)
    hc = np.asarray(h_center, np.float32)
    mask = np.asarray(neighbor_mask)
    W1 = np.asarray(W1, np.float32)
    w1s = (W1[:H] + W1[H:]).astype(bf16)
    w1b = W1[H:].astype(bf16)
    wq = (np.asarray(Wq, np.float32) / np.sqrt(DH)).astype(bf16)
    wk = np.asarray(Wk, bf16)
    wv = np.asarray(Wv, bf16)
    wo = np.asarray(Wo, bf16)

    # [m, ct, f, n, c] feature-major neighbor tensor
    hnf = np.ascontiguousarray(
        hn.reshape(M, CT, P, N, H).transpose(0, 1, 4, 3, 2)).astype(bf16)
    hcf = np.ascontiguousarray(
        hc.reshape(M, BL, H).transpose(0, 2, 1)).astype(bf16)
    mb = np.where(mask, 0.0, NEG).astype(np.float32).reshape(M, CT, P, N)

    in_maps = []
    for m in range(M):
        in_maps.append({
            "hnf": np.ascontiguousarray(hnf[m]),
            "hcf": np.ascontiguousarray(hcf[m]),
            "mb": np.ascontiguousarray(mb[m]),
            "w1s": w1s, "w1b": w1b, "wq": wq, "wk": wk, "wv": wv, "wo": wo,
        })
    return in_maps


def _fast_path_ok(b1, ln_g, ln_b, bq, bk, bv, bo):
    return (np.all(np.asarray(b1) == 0) and np.all(np.asarray(ln_g) == 1)
            and np.all(np.asarray(ln_b) == 0) and np.all(np.asarray(bq) == 0)
            and np.all(np.asarray(bv) == 0) and np.all(np.asarray(bo) == 0))


def _np_fallback(h_center, h_neighbors, W1, b1, ln_g, ln_b, Wq, bq, Wk, bk,
                 Wv, bv, Wo, bo, neighbor_mask):
    from scipy.special import erf

    hc = np.asarray(h_center, np.float32)
    hn = np.asarray(h_neighbors, np.float32)
    diff = hn - hc[:, None, :]
    comb = np.concatenate([hn, diff], -1)
    pre = comb @ W1 + b1
    mu = pre.mean(-1, keepdims=True)
    var = ((pre - mu) ** 2).mean(-1, keepdims=True)
    x = (pre - mu) / np.sqrt(var + LN_EPS) * ln_g + ln_b
    hnp_ = 0.5 * x * (1 + erf(x / np.sqrt(2)))
    Q = (hc @ Wq + bq).reshape(B, NHEAD, DH)
    K = (hnp_ @ Wk + bk).reshape(B, N, NHEAD, DH)
    V = (hnp_ @ Wv + bv).reshape(B, N, NHEAD, DH)
    sc = np.einsum("bhd,bnhd->bhn", Q, K) / np.sqrt(DH)
    sc = np.where(neighbor_mask[:, None, :], sc, -np.inf)
    sc = sc - sc.max(-1, keepdims=True)
    e = np.exp(sc)
    attn = e / e.sum(-1, keepdims=True)
    ctx = np.einsum("bhn,bnhd->bhd", attn, V).reshape(B, H)
    return (ctx @ Wo + bo).astype(np.float32)


def run_spmd(in_maps, **kwargs):
    from concourse.bass_utils import run_bass_kernel_spmd

    return run_bass_kernel_spmd(_get_nc(), in_maps, core_ids=list(range(M)),
                                **kwargs)


def kernel(h_center, h_neighbors, W1, b1, ln_g, ln_b, Wq, bq, Wk, bk, Wv, bv,
           Wo, bo, neighbor_mask):
    if not _fast_path_ok(b1, ln_g, ln_b, bq, bk, bv, bo):
        return _np_fallback(h_center, h_neighbors, W1, b1, ln_g, ln_b, Wq, bq,
                            Wk, bk, Wv, bv, Wo, bo, neighbor_mask)
    in_maps = _pack_inputs(h_center, h_neighbors, neighbor_mask, W1, Wq, Wk,
                           Wv, Wo)
    res = run_spmd(in_maps)
    return np.concatenate([r["out"] for r in res.results],
                          axis=0).astype(np.float32)
